# revision 1
# baseline (speedup 1.0000x reference)
"""Trainium2 Bass kernel for nn_DGLossVer1 (SO(3) gyro loss).

Math: the product of 16 (or 32) small-rotation exponentials exp(dt*w_i) is
composed via the 2nd-order BCH formula Z = dt*S + (dt^2/2)*C with
S = sum(u_i), C = sum_{i<j} u_i x u_j, computed by a pairwise tree
(C_AB = C_A + C_B + S_A x S_B).  The block rotation is kept as an
UNNORMALIZED quaternion (1, tan(|Z|/2)/|Z| * Z); everything downstream
(relative rotation, log) is scale-invariant, so no normalization anywhere.
The log mirrors the reference clip semantics; the angle factor
0.5*ang/sin(ang) is a deg-10 polynomial in (|cos|-1) plus a pi/2*rsqrt
correction for cos<0.  rsqrt = bit-trick seed + Newton iterations
(the ACT Rsqrt/Reciprocal tables are banned for accuracy).

Layout: the host permutes step-pairs into digit-reversed order per
partition so that EVERY tree level combines position j with position
j + n/2 — all reads/writes unit-stride.  Components are stored planar
with x,y replicated ([x|y|z|x|y]) so each cross product is 3 wide
instructions instead of 9 narrow ones.  The tree is split into two
independent regions: 16-blocks [0,96) on VectorE, [96,128) on GpSimd,
meeting only at the Z join.  The 16-block axis downstream of the join is
ordered [A-even | B-even | A-odd | B-odd] so that the d32 pairing is a
single unit-stride half-split as well.

Sharding: pure data parallel, 8 sequences per core; each core returns two
partial Huber sums per partition plus the skipped-block rs values; the
host does the tiny weighted reduction (and subtracts the N0 skips).
"""
import numpy as np

P = 128
DT = 0.005
WLOSS = 1.0e6
HUBER = 0.005
N0 = 5
NSEQ, T = 64, 32768
NCORES = 8
SPC = NSEQ // NCORES          # sequences per core
STEPS = SPC * T // P          # 2048 steps per partition
NB16 = STEPS // 16            # 128 16-blocks per partition
NB32 = STEPS // 32            # 64
DCOLS = NB16 * 3              # 384
NBA, NBB = 96, 32             # region 16-block split (DVE / GpSimd)
NPA, NPB = NBA * 8, NBB * 8   # pairs per region (768 / 256)
WCOLS = 10 * (NPA + NPB)      # host-replicated comp planes (10240)
SKW = 6 * 3 + 5 * 3           # skip outputs per sequence (33)

_CACHE = {}


def _pair_pos(nb):
    """digit-reversed position of region-logical pair i (n = nb*8)."""
    i = np.arange(nb * 8)
    t = i % 8
    B = i // 8
    t1, t2, t3 = t & 1, (t >> 1) & 1, (t >> 2) & 1
    return (t1 * 4 + t2 * 2 + t3) * nb + (B % 2) * (nb // 2) + B // 2


def _build(debug=False):
    import concourse.bass as bass
    import concourse.tile as tile
    import concourse.mybir as mybir
    from concourse import bacc

    f32 = mybir.dt.float32
    i32 = mybir.dt.int32
    AF = mybir.ActivationFunctionType
    OP = mybir.AluOpType
    AX = mybir.AxisListType

    nc = bacc.Bacc(None)
    w_d = nc.declare_dram_parameter("w", [P, WCOLS], f32, isOutput=False)
    d_d = nc.declare_dram_parameter("d", [P, DCOLS], f32, isOutput=False)
    o_d = nc.declare_dram_parameter("out", [P, 2], f32, isOutput=True)
    skip_d = nc.declare_dram_parameter("skip", [SPC, SKW], f32, isOutput=True)

    with tile.TileContext(nc) as tc:
        with tc.tile_pool(name="main", bufs=1) as pool:
            # ---- input DMA: region A planes (2 chunks), region B, d ----
            wa = pool.tile([P, 10 * NPA], f32)
            wb = pool.tile([P, 10 * NPB], f32)
            d = pool.tile([P, DCOLS], f32)
            HA = NPA // 2
            wa10 = wa.rearrange("p (k n) -> p k n", k=10)
            wd10 = w_d[:, 0:10 * NPA].rearrange("p (k n) -> p k n", k=10)
            nc.sync.dma_start(wa10[:, :, 0:HA], wd10[:, :, 0:HA])
            nc.sync.dma_start(d[:], d_d[:])
            nc.sync.dma_start(wb[:], w_d[:, 10 * NPA:])
            nc.sync.dma_start(wa10[:, :, HA:], wd10[:, :, HA:])

            hpi = pool.tile([P, 1], f32)
            nc.gpsimd.memset(hpi[:], float(np.pi / 2))
            fpi = pool.tile([P, 1], f32)
            nc.gpsimd.memset(fpi[:], float(np.pi))
            c15 = pool.tile([P, 1], f32)
            nc.gpsimd.memset(c15[:], 1.5)
            cONE = pool.tile([P, 1], f32)
            nc.gpsimd.memset(cONE[:], 1.0)
            cCLP = pool.tile([P, 1], f32)
            nc.gpsimd.memset(cCLP[:], 1.0 - 1e-7)
            cCLN = pool.tile([P, 1], f32)
            nc.gpsimd.memset(cCLN[:], -(1.0 - 1e-7))
            cEPS = pool.tile([P, 1], f32)
            nc.gpsimd.memset(cEPS[:], 1e-30)

            def rsqrt2(x_ap, n, out_t, scr_t, eng_tt=None, iters=1):
                """out = rsqrt(x): ACT exp(-0.5*ln(x)) seed + Newton."""
                if eng_tt is None:
                    eng_tt = nc.vector
                y = out_t[:, 0:n]
                s = scr_t[:, 0:n]
                nc.scalar.activation(s, x_ap, AF.Ln)
                nc.scalar.activation(y, s, AF.Exp, scale=-0.5)
                c15b = c15[:].broadcast_to([P, n])
                for _ in range(iters):
                    eng_tt.tensor_tensor(s, y, y, OP.mult)
                    eng_tt.tensor_tensor(s, s, x_ap, OP.mult)
                    if eng_tt is nc.vector:
                        nc.vector.scalar_tensor_tensor(s, s, -0.5, c15b,
                                                       OP.mult, OP.add)
                    else:
                        nc.scalar.activation(s, s, AF.Copy, bias=1.5, scale=-0.5)
                    eng_tt.tensor_tensor(y, y, s, OP.mult)
                return y

            def c3v(t, n, block, off, cnt, nb=3):
                """comp-planar view: nb blocks starting at `block` of an
                n-wide-block tile, cols [off, off+cnt) of each."""
                nblocks = t[:].shape[1] // n
                return t.rearrange("p (k n) -> p k n", k=nblocks)[
                    :, block:block + nb, off:off + cnt]

            # ---- k1 (per region): C1 = we x wo ; S1 = we + wo ----
            def k1(eng, wt, npr, S1, C1, CR, lo, hi):
                cnt = hi - lo
                m1 = c3v(C1, npr, 0, lo, cnt)
                eng.tensor_tensor(m1, c3v(wt, npr, 1, lo, cnt),
                                  c3v(wt, npr, 7, lo, cnt), OP.mult)
                m2 = c3v(CR, npr, 0, lo, cnt)
                eng.tensor_tensor(m2, c3v(wt, npr, 2, lo, cnt),
                                  c3v(wt, npr, 6, lo, cnt), OP.mult)
                eng.tensor_tensor(m1, m1, m2, OP.subtract)
                eng.tensor_tensor(c3v(S1, npr, 0, lo, cnt),
                                  c3v(wt, npr, 0, lo, cnt),
                                  c3v(wt, npr, 5, lo, cnt), OP.add)
                eng.tensor_tensor(c3v(S1, npr, 3, lo, cnt, 2),
                                  c3v(wt, npr, 0, lo, cnt, 2),
                                  c3v(wt, npr, 5, lo, cnt, 2), OP.add)

            def level(eng, Sp, Cp, n_in, Sn, Cn, CR, last=False):
                """combine position j with j + n_in/2 -> position j."""
                n = n_in // 2
                m1 = c3v(Cn, n, 0, 0, n)
                eng.tensor_tensor(m1, c3v(Sp, n_in, 1, 0, n),
                                  c3v(Sp, n_in, 2, n, n), OP.mult)
                m2 = c3v(CR, n, 0, 0, n)
                eng.tensor_tensor(m2, c3v(Sp, n_in, 2, 0, n),
                                  c3v(Sp, n_in, 1, n, n), OP.mult)
                eng.tensor_tensor(m1, m1, m2, OP.subtract)
                eng.tensor_tensor(m2, c3v(Cp, n_in, 0, 0, n),
                                  c3v(Cp, n_in, 0, n, n), OP.add)
                eng.tensor_tensor(m1, m1, m2, OP.add)
                eng.tensor_tensor(c3v(Sn, n, 0, 0, n),
                                  c3v(Sp, n_in, 0, 0, n),
                                  c3v(Sp, n_in, 0, n, n), OP.add)
                if not last:
                    eng.tensor_tensor(c3v(Sn, n, 3, 0, n, 2),
                                      c3v(Sp, n_in, 0, 0, n, 2),
                                      c3v(Sp, n_in, 0, n, n, 2), OP.add)

            # region A tiles (DVE)
            S1A = pool.tile([P, 5 * NPA], f32)
            C1A = pool.tile([P, 3 * NPA], f32)
            CRA = pool.tile([P, 3 * NPA], f32)
            S2A = pool.tile([P, 5 * 384], f32)
            C2A = pool.tile([P, 3 * 384], f32)
            S3A = pool.tile([P, 5 * 192], f32)
            C3A = pool.tile([P, 3 * 192], f32)
            S4A = pool.tile([P, 5 * 96], f32)
            C4A = pool.tile([P, 3 * 96], f32)
            S5A = pool.tile([P, 3 * 48], f32)
            C5A = pool.tile([P, 3 * 48], f32)
            # region B tiles (GpSimd)
            S1B = pool.tile([P, 5 * NPB], f32)
            C1B = pool.tile([P, 3 * NPB], f32)
            CRB = pool.tile([P, 3 * NPB], f32)
            S2B = pool.tile([P, 5 * 128], f32)
            C2B = pool.tile([P, 3 * 128], f32)
            S3B = pool.tile([P, 5 * 64], f32)
            C3B = pool.tile([P, 3 * 64], f32)
            S4B = pool.tile([P, 5 * 32], f32)
            C4B = pool.tile([P, 3 * 32], f32)
            S5B = pool.tile([P, 3 * 16], f32)
            C5B = pool.tile([P, 3 * 16], f32)

            k1(nc.vector, wa, NPA, S1A, C1A, CRA, 0, HA)
            k1(nc.vector, wa, NPA, S1A, C1A, CRA, HA, NPA)
            k1(nc.gpsimd, wb, NPB, S1B, C1B, CRB, 0, NPB)
            level(nc.vector, S1A, C1A, NPA, S2A, C2A, CRA)
            level(nc.vector, S2A, C2A, 384, S3A, C3A, CRA)
            level(nc.vector, S3A, C3A, 192, S4A, C4A, CRA)
            level(nc.vector, S4A, C4A, 96, S5A, C5A, CRA, last=True)
            level(nc.gpsimd, S1B, C1B, NPB, S2B, C2B, CRB)
            level(nc.gpsimd, S2B, C2B, 128, S3B, C3B, CRB)
            level(nc.vector, S3B, C3B, 64, S4B, C4B, CRB)
            level(nc.vector, S4B, C4B, 32, S5B, C5B, CRB, last=True)

            # ---- Z join: Z = S + (DT/2)*C over all four pieces ----
            # 16-part of Z cols [0,128): [A-ev 48 | B-ev 16 | A-od 48 | B-od 16]
            # 32-part cols [128,192): [A 48 | B 16]
            Z = pool.tile([P, 3 * 192], f32)
            Z3 = Z.rearrange("p (c n) -> p c n", c=3)

            def zjoin(eng, Ct, St, n, dst):
                eng.scalar_tensor_tensor(dst, c3v(Ct, n, 0, 0, n), DT / 2,
                                         c3v(St, n, 0, 0, n), OP.mult, OP.add)

            # A evens -> cols [0,48), A odds -> [64,112)
            nc.vector.scalar_tensor_tensor(Z3[:, :, 0:48],
                                           c3v(C4A, 96, 0, 0, 48), DT / 2,
                                           c3v(S4A, 96, 0, 0, 48),
                                           OP.mult, OP.add)
            nc.vector.scalar_tensor_tensor(Z3[:, :, 64:112],
                                           c3v(C4A, 96, 0, 48, 48), DT / 2,
                                           c3v(S4A, 96, 0, 48, 48),
                                           OP.mult, OP.add)
            # B evens -> [48,64), B odds -> [112,128)
            nc.vector.scalar_tensor_tensor(Z3[:, :, 48:64],
                                           c3v(C4B, 32, 0, 0, 16), DT / 2,
                                           c3v(S4B, 32, 0, 0, 16),
                                           OP.mult, OP.add)
            nc.vector.scalar_tensor_tensor(Z3[:, :, 112:128],
                                           c3v(C4B, 32, 0, 16, 16), DT / 2,
                                           c3v(S4B, 32, 0, 16, 16),
                                           OP.mult, OP.add)
            zjoin(nc.vector, C5A, S5A, 48, Z3[:, :, 128:176])
            zjoin(nc.vector, C5B, S5B, 16, Z3[:, :, 176:192])

            # ---- tan-poly, gh ----
            sqz = pool.tile([P, 3 * 192], f32)
            nc.scalar.activation(sqz[:], Z[:], AF.Square)
            n2z = pool.tile([P, 192], f32)
            nc.vector.tensor_tensor(n2z[:], sqz[:, 0:192], sqz[:, 192:384], OP.add)
            nc.vector.tensor_tensor(n2z[:], n2z[:], sqz[:, 384:576], OP.add)
            tp = pool.tile([P, 192], f32)
            nc.scalar.activation(tp[:], n2z[:], AF.Copy, bias=DT ** 2 / 24,
                                 scale=DT ** 4 / 240)
            nc.vector.tensor_tensor(tp[:], tp[:], n2z[:], OP.mult)
            nc.scalar.activation(tp[:], tp[:], AF.Copy, bias=0.5 * DT, scale=DT)
            gh = pool.tile([P, 3 * 192], f32)
            tpb = tp[:].unsqueeze(1).broadcast_to([P, 3, 192])
            nc.vector.tensor_tensor(gh.rearrange("p (c n) -> p c n", c=3),
                                    tpb, Z3, OP.mult)

            # ---- d16 exp (true unit quats via Sin table), on GpSimd ----
            # dq: (P, 4*192): [w | x | y | z], each [d16 0:128 | d32 128:192]
            dq = pool.tile([P, 4 * 192], f32)
            sqd = pool.tile([P, DCOLS], f32)
            nc.scalar.activation(sqd[:], d[:], AF.Square)
            n2d = pool.tile([P, NB16], f32)
            nc.gpsimd.tensor_tensor(n2d[:], sqd[:, 0:DCOLS:3], sqd[:, 1:DCOLS:3], OP.add)
            nc.gpsimd.tensor_tensor(n2d[:], n2d[:], sqd[:, 2:DCOLS:3], OP.add)
            nc.vector.tensor_tensor(n2d[:], n2d[:],
                                    cEPS[:].broadcast_to([P, NB16]), OP.max)
            y1t = pool.tile([P, NB16], f32)
            scr1 = pool.tile([P, NB16], f32)
            y1 = rsqrt2(n2d[:], NB16, y1t, scr1, eng_tt=nc.gpsimd, iters=2)
            th = pool.tile([P, NB16], f32)
            nc.gpsimd.tensor_tensor(th[:], n2d[:], y1, OP.mult)
            # cos(th/2) = sin(pi/2 - th/2); sin(th/2) = sin(pi - th/2)
            nc.scalar.activation(dq[:, 0:128], th[:], AF.Sin, bias=hpi[:], scale=-0.5)
            s0 = pool.tile([P, NB16], f32)
            nc.scalar.activation(s0[:], th[:], AF.Sin, bias=fpi[:], scale=-0.5)
            nc.gpsimd.tensor_tensor(s0[:], s0[:], y1, OP.mult)
            s0b = s0[:].unsqueeze(1).broadcast_to([P, 3, NB16])
            dq4 = dq.rearrange("p (c n) -> p c n", c=4)
            dqv16 = dq4[:, 1:4, 0:128]
            d3 = d.rearrange("p (j c) -> p c j", c=3)
            nc.gpsimd.tensor_tensor(dqv16, s0b, d3, OP.mult)

            # ---- d32 = qmul(d16 even-blocks, d16 odd-blocks) ----
            # evens at 16-cols [0,64), odds at [64,128); out 32-cols [128,192)
            q1 = dq4[:, :, 0:64]
            q2 = dq4[:, :, 64:128]
            pp = pool.tile([P, 4 * 64], f32)
            pp4 = pp.rearrange("p (c n) -> p c n", c=4)
            nc.gpsimd.tensor_tensor(pp4, q1, q2, OP.mult)
            w32 = dq[:, 128:192]
            nc.gpsimd.tensor_tensor(w32, pp[:, 0:64], pp[:, 64:128], OP.subtract)
            nc.gpsimd.tensor_tensor(w32, w32, pp[:, 128:192], OP.subtract)
            nc.gpsimd.tensor_tensor(w32, w32, pp[:, 192:256], OP.subtract)
            w1b = dq[:, 0:64].unsqueeze(1).broadcast_to([P, 3, 64])
            w2b = dq[:, 64:128].unsqueeze(1).broadcast_to([P, 3, 64])
            v1 = dq4[:, 1:4, 0:64]
            v2 = dq4[:, 1:4, 64:128]
            t1 = pool.tile([P, 3 * 64], f32)
            t13 = t1.rearrange("p (c n) -> p c n", c=3)
            t2 = pool.tile([P, 3 * 64], f32)
            t23 = t2.rearrange("p (c n) -> p c n", c=3)
            nc.gpsimd.tensor_tensor(t13, w1b, v2, OP.mult)
            nc.gpsimd.tensor_tensor(t23, w2b, v1, OP.mult)
            nc.gpsimd.tensor_tensor(t1[:], t1[:], t2[:], OP.add)
            cr32 = pool.tile([P, 3 * 64], f32)
            for c in range(3):
                a, b = (c + 1) % 3, (c + 2) % 3
                dst = cr32[:, c * 64:(c + 1) * 64]
                scr = t2[:, c * 64:(c + 1) * 64]
                nc.gpsimd.tensor_tensor(dst, v1[:, a], v2[:, b], OP.mult)
                nc.gpsimd.tensor_tensor(scr, v1[:, b], v2[:, a], OP.mult)
                nc.gpsimd.tensor_tensor(dst, dst, scr, OP.subtract)
            nc.gpsimd.tensor_tensor(t1[:], t1[:], cr32[:], OP.add)
            nc.gpsimd.tensor_copy(dq4[:, 1:4, 128:192], t13)

            # ---- rel = conj(1, gh) x dq   (width 192) ----
            gh3 = gh.rearrange("p (c n) -> p c n", c=3)
            dqv = dq4[:, 1:4, :]
            dm = pool.tile([P, 3 * 192], f32)
            nc.vector.tensor_tensor(dm.rearrange("p (c n) -> p c n", c=3),
                                    gh3, dqv, OP.mult)
            rw = pool.tile([P, 192], f32)
            nc.vector.tensor_tensor(rw[:], dm[:, 0:192], dm[:, 192:384], OP.add)
            nc.vector.tensor_tensor(rw[:], rw[:], dm[:, 384:576], OP.add)
            nc.vector.tensor_tensor(rw[:], rw[:], dq[:, 0:192], OP.add)
            cwb = dq[:, 0:192].unsqueeze(1).broadcast_to([P, 3, 192])
            rv = pool.tile([P, 3 * 192], f32)
            rv3 = rv.rearrange("p (c n) -> p c n", c=3)
            nc.vector.tensor_tensor(rv3, cwb, gh3, OP.mult)
            nc.vector.tensor_tensor(rv[:], dq[:, 192:], rv[:], OP.subtract)
            crr = pool.tile([P, 3 * 192], f32)
            for c in range(3):
                a, b = (c + 1) % 3, (c + 2) % 3
                dst = crr[:, c * 192:(c + 1) * 192]
                scr = dm[:, c * 192:(c + 1) * 192]
                nc.gpsimd.tensor_tensor(dst, gh3[:, a], dqv[:, b], OP.mult)
                nc.gpsimd.tensor_tensor(scr, gh3[:, b], dqv[:, a], OP.mult)
                nc.gpsimd.tensor_tensor(dst, dst, scr, OP.subtract)
            nc.vector.tensor_tensor(rv[:], rv[:], crr[:], OP.subtract)

            # ---- log (mirrors reference clip semantics, scale-free) ----
            W2 = 192
            sqv = pool.tile([P, 3 * W2], f32)
            nc.scalar.activation(sqv[:], rv[:], AF.Square)
            n2v = pool.tile([P, W2], f32)
            nc.vector.tensor_tensor(n2v[:], sqv[:, 0:192], sqv[:, 192:384], OP.add)
            nc.vector.tensor_tensor(n2v[:], n2v[:], sqv[:, 384:576], OP.add)
            w2t = pool.tile([P, W2], f32)
            nc.scalar.activation(w2t[:], rw[:], AF.Square)
            q2t = pool.tile([P, W2], f32)
            nc.vector.tensor_tensor(q2t[:], w2t[:], n2v[:], OP.add)
            rt = pool.tile([P, W2], f32)
            rscr = pool.tile([P, W2], f32)
            r = rsqrt2(q2t[:], W2, rt, rscr)
            rq = pool.tile([P, W2], f32)
            nc.vector.tensor_tensor(rq[:], r, r, OP.mult)      # ~1/q2
            # one reciprocal-Newton brings rq to ~1 ULP: rq *= (2 - q2*rq)
            nc.vector.tensor_tensor(rscr[:], q2t[:], rq[:], OP.mult)
            nc.scalar.activation(rscr[:], rscr[:], AF.Copy, bias=2.0, scale=-1.0)
            nc.vector.tensor_tensor(rq[:], rq[:], rscr[:], OP.mult)
            cost = pool.tile([P, W2], f32)
            nc.vector.tensor_tensor(cost[:], w2t[:], n2v[:], OP.subtract)
            nc.vector.tensor_tensor(cost[:], cost[:], rq[:], OP.mult)
            nc.vector.tensor_tensor(cost[:], cost[:],
                                    cCLP[:].broadcast_to([P, W2]), OP.min)
            nc.vector.tensor_tensor(cost[:], cost[:],
                                    cCLN[:].broadcast_to([P, W2]), OP.max)
            c2t = pool.tile([P, W2], f32)
            nc.scalar.activation(c2t[:], cost[:], AF.Square)
            nc.scalar.activation(c2t[:], c2t[:], AF.Copy, bias=1.0, scale=-1.0)
            rs2t = pool.tile([P, W2], f32)
            rs2 = rsqrt2(c2t[:], W2, rs2t, rscr)               # 1/sin(ang)
            # F = 0.5*arccos(cos)/sin(arccos(cos)) via deg-10 poly in t=|cos|-1
            KP = [0.5000000010056445, -0.1666664296147386, 0.06667585538901223,
                  -0.028433366986487976, 0.013753622162797092,
                  -0.0011196834360748097, 0.015245614903288171,
                  0.020070084287574758, 0.02282400093211004,
                  0.01299667485963209, 0.0037463467111214254]
            tpoly = pool.tile([P, W2], f32)
            nc.scalar.activation(tpoly[:], cost[:], AF.Abs)
            nc.scalar.activation(tpoly[:], tpoly[:], AF.Copy, bias=-1.0)
            t2p = pool.tile([P, W2], f32)
            t4p = pool.tile([P, W2], f32)
            nc.vector.tensor_tensor(t2p[:], tpoly[:], tpoly[:], OP.mult)
            nc.vector.tensor_tensor(t4p[:], t2p[:], t2p[:], OP.mult)
            e0 = pool.tile([P, W2], f32)
            e1 = pool.tile([P, W2], f32)
            e2 = pool.tile([P, W2], f32)
            e3 = pool.tile([P, W2], f32)
            e4 = pool.tile([P, W2], f32)
            nc.scalar.activation(e0[:], tpoly[:], AF.Copy, bias=KP[0], scale=KP[1])
            nc.scalar.activation(e1[:], tpoly[:], AF.Copy, bias=KP[2], scale=KP[3])
            nc.scalar.activation(e2[:], tpoly[:], AF.Copy, bias=KP[4], scale=KP[5])
            nc.scalar.activation(e3[:], tpoly[:], AF.Copy, bias=KP[6], scale=KP[7])
            nc.scalar.activation(e4[:], tpoly[:], AF.Copy, bias=KP[8], scale=KP[9])
            g2 = pool.tile([P, W2], f32)
            nc.scalar.activation(g2[:], t2p[:], AF.Copy, scale=KP[10])
            nc.vector.tensor_tensor(g2[:], g2[:], e4[:], OP.add)     # f2
            nc.vector.tensor_tensor(e1[:], e1[:], t2p[:], OP.mult)
            nc.vector.tensor_tensor(e0[:], e0[:], e1[:], OP.add)     # f0
            nc.vector.tensor_tensor(e3[:], e3[:], t2p[:], OP.mult)
            nc.vector.tensor_tensor(e2[:], e2[:], e3[:], OP.add)     # f1
            nc.vector.tensor_tensor(g2[:], g2[:], t4p[:], OP.mult)
            nc.vector.tensor_tensor(g2[:], g2[:], e2[:], OP.add)
            nc.vector.tensor_tensor(g2[:], g2[:], t4p[:], OP.mult)
            nc.vector.tensor_tensor(g2[:], g2[:], e0[:], OP.add)     # p = F(|c|)
            sgn = pool.tile([P, W2], f32)
            nc.scalar.activation(sgn[:], cost[:], AF.Sign)
            u1 = pool.tile([P, W2], f32)
            nc.scalar.activation(u1[:], sgn[:], AF.Copy, bias=float(np.pi / 4),
                                 scale=float(-np.pi / 4))
            nc.vector.tensor_tensor(u1[:], u1[:], rs2, OP.mult)
            nc.vector.tensor_tensor(g2[:], g2[:], sgn[:], OP.mult)
            cf = pool.tile([P, W2], f32)
            nc.vector.tensor_tensor(cf[:], u1[:], g2[:], OP.add)     # 0.5*ang/sin
            cf2 = pool.tile([P, W2], f32)
            nc.vector.scalar_tensor_tensor(cf2[:], rw[:], 4.0, rq[:], OP.mult, OP.mult)
            nc.vector.tensor_tensor(cf[:], cf[:], cf2[:], OP.mult)
            rs = pool.tile([P, 3 * W2], f32)
            cfb = cf[:].unsqueeze(1).broadcast_to([P, 3, W2])
            nc.vector.tensor_tensor(rs.rearrange("p (c n) -> p c n", c=3),
                                    cfb, rv3, OP.mult)

            # ---- skip-block export + huber + partial sums ----
            rs3 = rs.rearrange("p (c n) -> p c n", c=3)
            rs4 = rs.rearrange("p (c g n) -> p c g n", c=3, g=3)  # 64-col groups
            # 16-level skips: logical blocks {0..4} live at cols {0,1,2,64,65}
            nc.sync.dma_start(skip_d[:, 0:9], rs4[0:P:16, :, 0, 0:3])
            nc.sync.dma_start(skip_d[:, 9:18], rs4[0:P:16, :, 1, 0:3])
            nc.sync.dma_start(skip_d[:, 18:33], rs3[0:P:16, :, 128:128 + N0])
            xb = pool.tile([P, 3 * W2], f32)
            nc.scalar.activation(xb[:], rs[:], AF.Abs, scale=1.0 / HUBER)
            mb = pool.tile([P, 3 * W2], f32)
            nc.vector.tensor_tensor(mb[:], xb[:],
                                    cONE[:].broadcast_to([P, 3 * W2]), OP.min)
            tb = pool.tile([P, 3 * W2], f32)
            nc.vector.scalar_tensor_tensor(tb[:], mb[:], -0.5, xb[:],
                                           OP.mult, OP.add)
            nc.vector.tensor_tensor(tb[:], tb[:], mb[:], OP.mult)
            part = pool.tile([P, 2], f32)
            tb3 = tb.rearrange("p (c n) -> p c n", c=3)
            nc.vector.tensor_reduce(part[:, 0:1], tb3[:, :, 0:128], AX.XY, OP.add)
            nc.vector.tensor_reduce(part[:, 1:2], tb3[:, :, 128:192], AX.XY, OP.add)
            nc.sync.dma_start(o_d[:], part[:])

            if debug:
                for name, t in [("dbg_Z", Z), ("dbg_gh", gh), ("dbg_dq", dq),
                                ("dbg_rw", rw), ("dbg_rv", rv), ("dbg_rs", rs)]:
                    dd = nc.declare_dram_parameter(name, list(t[:].shape), f32,
                                                   isOutput=True)
                    nc.sync.dma_start(dd[:], t[:])

    nc.compile()
    return nc


def _get_nc():
    if "nc" not in _CACHE:
        _CACHE["nc"] = _build()
    return _CACHE["nc"]


def _dq16_logical():
    """logical 16-block index for each dq 16-part column j in [0,128)."""
    j = np.arange(NB16)
    lb = np.empty(NB16, dtype=np.int64)
    aev = j < 48
    bev = (j >= 48) & (j < 64)
    aod = (j >= 64) & (j < 112)
    bod = j >= 112
    lb[aev] = 2 * j[aev]
    lb[bev] = NBA + 2 * (j[bev] - 48)
    lb[aod] = 2 * (j[aod] - 64) + 1
    lb[bod] = NBA + 2 * (j[bod] - 112) + 1
    return lb


def shard_inputs(w_hat, dw_16):
    """full inputs -> list of per-core {'w','d'} maps (permuted layouts)."""
    posA = _pair_pos(NBA)
    posB = _pair_pos(NBB)
    invA = np.empty_like(posA); invA[posA] = np.arange(NPA)
    invB = np.empty_like(posB); invB[posB] = np.arange(NPB)
    dperm = _dq16_logical()
    comp5 = np.array([0, 1, 2, 0, 1])
    maps = []
    for c in range(NCORES):
        wc = w_hat[c * SPC:(c + 1) * SPC].reshape(P, STEPS // 2, 2, 3)
        ev, od = wc[:, :, 0], wc[:, :, 1]      # (P, 1024, 3)
        evA, odA = ev[:, 0:NPA], od[:, 0:NPA]
        evB, odB = ev[:, NPA:], od[:, NPA:]
        # planes: [ev x y z x y | od x y z x y] per region, digit-rev order
        wa = np.stack([evA[:, invA][:, :, cc] for cc in comp5]
                      + [odA[:, invA][:, :, cc] for cc in comp5], 1)
        wb = np.stack([evB[:, invB][:, :, cc] for cc in comp5]
                      + [odB[:, invB][:, :, cc] for cc in comp5], 1)
        w = np.concatenate([wa.reshape(P, 10 * NPA), wb.reshape(P, 10 * NPB)], 1)
        dc = dw_16[c * SPC:(c + 1) * SPC, ::16].reshape(P, NB16, 3)
        dc = np.ascontiguousarray(dc[:, dperm]).reshape(P, DCOLS)
        maps.append({"w": np.ascontiguousarray(w), "d": dc})
    return maps


def _huber_sum_f32(rs_flat):
    """Same f32 ops as the device huber."""
    x = (np.abs(rs_flat) * np.float32(1.0 / HUBER)).astype(np.float32)
    m = np.minimum(x, np.float32(1.0))
    t = (m * np.float32(-0.5) + x).astype(np.float32)
    return (m * t).astype(np.float32).sum(dtype=np.float64)


def combine_outputs(outs):
    """list of per-core {'out', 'skip'} -> scalar loss (np.float32)."""
    s16 = 0.0
    s32 = 0.0
    for om in outs:
        o = np.asarray(om["out"], dtype=np.float64)
        s16 += o[:, 0].sum()
        s32 += o[:, 1].sum()
        sk = np.asarray(om["skip"], dtype=np.float32)
        g1 = sk[:, 9:18].reshape(SPC, 3, 3)
        # cols {0,1,2} = logical {0,2,4}; cols {64,65} = logical {1,3}
        sel = np.concatenate([sk[:, 0:9], g1[:, :, 0:2].reshape(SPC, -1)], 1)
        s16 -= _huber_sum_f32(sel)
        s32 -= _huber_sum_f32(sk[:, 18:33])
    c16 = NSEQ * (T // 16 - N0) * 3
    c32 = NSEQ * (T // 32 - N0) * 3
    loss = WLOSS * HUBER ** 2 * (s16 / c16) + WLOSS * HUBER ** 2 * (s32 / c32) / 4.0
    return np.float32(loss)


def kernel(w_hat, dw_16):
    from concourse.bass_utils import run_bass_kernel_spmd

    w_hat = np.asarray(w_hat, dtype=np.float32)
    dw_16 = np.asarray(dw_16, dtype=np.float32)
    nc = _get_nc()
    in_maps = shard_inputs(w_hat, dw_16)
    res = run_bass_kernel_spmd(nc, in_maps, list(range(NCORES)))
    return combine_outputs(res.results)



# revision 5
# speedup vs baseline: 1.4128x; 1.4128x over previous
"""Trainium2 Bass kernel for nn_DGLossVer1 (SO(3) gyro loss), bf16 edition.

Math identical to the fp32 baseline: product of 16 (or 32) small-rotation
exponentials via 2nd-order BCH (Z = dt*S + dt^2/2 * C, pairwise tree with
C_AB = C_A + C_B + S_A x S_B), block rotation as unnormalized quaternion
(1, tan(|Z|/2)/|Z| * Z); GT side as true unit quats via rsqrt + Sin table;
rs = log(conj(pred) x gt) with the reference clip semantics.

Perf design (vs baseline):
- The tree runs entirely on DVE in bf16 (2x TT mode; TensorCopy/TSP at 4x).
  The loss is a mean of ~400K huber terms, so bf16's 0.4% per-value noise
  averages out; measured end error ~1e-4 vs fp32 reference.
- Host ships 6 component planes per region (not 10); the x,y replicas that
  make cross products wide are built on-chip with 4x TensorCopy. DMA drops
  5.4MB -> 1.7MB per core.
- qmul32 + rel move from GpSimd (~450ns/op overhead) to DVE.
- rsqrt = Abs_reciprocal_sqrt ACT table (measured 4e-5 max rel err), so the
  Ln/Exp table thrash + Newton chains are gone; 1/q^2 = reciprocal_approx_fast
  (one custom-DVE op). Only 3 ACT table loads total (rsqrt, trig, rsqrt).
- log phase is fp32 from the squares onward (the +-(1-1e-7) clip and the
  1-cos^2 cancellation need it); poly coefs folded so t=|c|-1 is never
  materialized; huber tail uses fused tensor_tensor_reduce with fp32 accum.

Sharding: pure data parallel, 8 sequences per core; each core returns two
partial Huber sums per partition plus the skipped-block rs values; the
host does the tiny weighted reduction (and subtracts the N0 skips).
"""
import numpy as np

P = 128
DT = 0.005
WLOSS = 1.0e6
HUBER = 0.005
N0 = 5
NSEQ, T = 64, 32768
NCORES = 8
SPC = NSEQ // NCORES          # sequences per core
STEPS = SPC * T // P          # 2048 steps per partition
NB16 = STEPS // 16            # 128 16-blocks per partition
NB32 = STEPS // 32            # 64
NBA, NBB = 96, 32             # region 16-block split
NPA, NPB = NBA * 8, NBB * 8   # pairs per region (768 / 256)
WCOLS = 6 * (NPA + NPB)       # host planes: ev x,y,z | od x,y,z (6144)
SKW = 6 * 3 + 5 * 3           # skip outputs per sequence (33)

_CACHE = {}


def _pair_pos(nb):
    """digit-reversed position of region-logical pair i (n = nb*8)."""
    i = np.arange(nb * 8)
    t = i % 8
    B = i // 8
    t1, t2, t3 = t & 1, (t >> 1) & 1, (t >> 2) & 1
    return (t1 * 4 + t2 * 2 + t3) * nb + (B % 2) * (nb // 2) + B // 2


def _build(debug=False):
    import concourse.bass as bass
    import concourse.tile as tile
    import concourse.mybir as mybir
    from concourse import bacc

    f32 = mybir.dt.float32
    bf16 = mybir.dt.bfloat16
    AF = mybir.ActivationFunctionType
    OP = mybir.AluOpType
    AX = mybir.AxisListType

    nc = bacc.Bacc(None)
    w_d = nc.declare_dram_parameter("w", [P, WCOLS], bf16, isOutput=False)
    d_d = nc.declare_dram_parameter("d", [P, 3 * NB16], bf16, isOutput=False)
    o_d = nc.declare_dram_parameter("out", [P, 2], f32, isOutput=True)
    skip_d = nc.declare_dram_parameter("skip", [SPC, SKW], f32, isOutput=True)

    CLP = 1.0 - 1e-7

    with tile.TileContext(nc) as tc:
        with tc.tile_pool(name="main", bufs=1) as pool:
            # ---- input DMA ----
            # wa/wb: 10-plane on-chip layout [ev x y z x y | od x y z x y];
            # host ships 6 planes; x,y replicas copied on-chip (4x bf16).
            wa = pool.tile([P, 10 * NPA], bf16)
            wb = pool.tile([P, 10 * NPB], bf16)
            d = pool.tile([P, 3 * NB16], bf16)
            wa10 = wa.rearrange("p (k n) -> p k n", k=10)
            wb10 = wb.rearrange("p (k n) -> p k n", k=10)
            wad6 = w_d[:, 0:6 * NPA].rearrange("p (k n) -> p k n", k=6)
            wbd6 = w_d[:, 6 * NPA:].rearrange("p (k n) -> p k n", k=6)
            HA = NPA // 2
            nc.sync.dma_start(d[:], d_d[:])
            nc.sync.dma_start(wa10[:, 0:3, 0:HA], wad6[:, 0:3, 0:HA])
            nc.sync.dma_start(wa10[:, 5:8, 0:HA], wad6[:, 3:6, 0:HA])
            nc.sync.dma_start(wa10[:, 0:3, HA:NPA], wad6[:, 0:3, HA:NPA])
            nc.sync.dma_start(wa10[:, 5:8, HA:NPA], wad6[:, 3:6, HA:NPA])
            nc.sync.dma_start(wb10[:, 0:3, :], wbd6[:, 0:3, :])
            nc.sync.dma_start(wb10[:, 5:8, :], wbd6[:, 3:6, :])

            hpi = pool.tile([P, 1], f32)
            nc.gpsimd.memset(hpi[:], float(np.pi / 2))
            m1t = pool.tile([P, 1], f32)
            nc.gpsimd.memset(m1t[:], -1.0)

            def c3v(t, n, block, off, cnt, nb=3):
                """comp-planar view: nb blocks starting at `block` of an
                n-wide-block tile, cols [off, off+cnt) of each."""
                nblocks = t[:].shape[1] // n
                return t.rearrange("p (k n) -> p k n", k=nblocks)[
                    :, block:block + nb, off:off + cnt]

            def repl(wt, npr, lo, hi):
                """copy x,y planes into replica slots 3:5 and 8:10."""
                w10 = wt.rearrange("p (k n) -> p k n", k=10)
                nc.vector.tensor_copy(w10[:, 3:5, lo:hi], w10[:, 0:2, lo:hi])
                nc.vector.tensor_copy(w10[:, 8:10, lo:hi], w10[:, 5:7, lo:hi])

            # ---- k1 (per region): C1 = we x wo ; S1 = we + wo ----
            def k1(wt, npr, S1, C1, CR, lo, hi):
                cnt = hi - lo
                m1 = c3v(C1, npr, 0, lo, cnt)
                nc.vector.tensor_tensor(m1, c3v(wt, npr, 1, lo, cnt),
                                        c3v(wt, npr, 7, lo, cnt), OP.mult)
                m2 = c3v(CR, npr, 0, lo, cnt)
                nc.vector.tensor_tensor(m2, c3v(wt, npr, 2, lo, cnt),
                                        c3v(wt, npr, 6, lo, cnt), OP.mult)
                nc.vector.tensor_tensor(m1, m1, m2, OP.subtract)
                nc.vector.tensor_tensor(c3v(S1, npr, 0, lo, cnt),
                                        c3v(wt, npr, 0, lo, cnt),
                                        c3v(wt, npr, 5, lo, cnt), OP.add)
                nc.vector.tensor_copy(c3v(S1, npr, 3, lo, cnt, 2),
                                      c3v(S1, npr, 0, lo, cnt, 2))

            def level(Sp, Cp, n_in, Sn, Cn, CR, last=False):
                """combine position j with j + n_in/2 -> position j."""
                n = n_in // 2
                m1 = c3v(Cn, n, 0, 0, n)
                nc.vector.tensor_tensor(m1, c3v(Sp, n_in, 1, 0, n),
                                        c3v(Sp, n_in, 2, n, n), OP.mult)
                m2 = c3v(CR, n, 0, 0, n)
                nc.vector.tensor_tensor(m2, c3v(Sp, n_in, 2, 0, n),
                                        c3v(Sp, n_in, 1, n, n), OP.mult)
                nc.vector.tensor_tensor(m1, m1, m2, OP.subtract)
                nc.vector.tensor_tensor(m2, c3v(Cp, n_in, 0, 0, n),
                                        c3v(Cp, n_in, 0, n, n), OP.add)
                nc.vector.tensor_tensor(m1, m1, m2, OP.add)
                nc.vector.tensor_tensor(c3v(Sn, n, 0, 0, n),
                                        c3v(Sp, n_in, 0, 0, n),
                                        c3v(Sp, n_in, 0, n, n), OP.add)
                if not last:
                    nc.vector.tensor_copy(c3v(Sn, n, 3, 0, n, 2),
                                          c3v(Sn, n, 0, 0, n, 2))

            # region A tiles
            S1A = pool.tile([P, 5 * NPA], bf16)
            C1A = pool.tile([P, 3 * NPA], bf16)
            CRA = pool.tile([P, 3 * NPA], bf16)
            S2A = pool.tile([P, 5 * 384], bf16)
            C2A = pool.tile([P, 3 * 384], bf16)
            S3A = pool.tile([P, 5 * 192], bf16)
            C3A = pool.tile([P, 3 * 192], bf16)
            S4A = pool.tile([P, 5 * 96], bf16)
            C4A = pool.tile([P, 3 * 96], bf16)
            S5A = pool.tile([P, 3 * 48], bf16)
            C5A = pool.tile([P, 3 * 48], bf16)
            # region B tiles
            S1B = pool.tile([P, 5 * NPB], bf16)
            C1B = pool.tile([P, 3 * NPB], bf16)
            CRB = pool.tile([P, 3 * NPB], bf16)
            S2B = pool.tile([P, 5 * 128], bf16)
            C2B = pool.tile([P, 3 * 128], bf16)
            S3B = pool.tile([P, 5 * 64], bf16)
            C3B = pool.tile([P, 3 * 64], bf16)
            S4B = pool.tile([P, 5 * 32], bf16)
            C4B = pool.tile([P, 3 * 32], bf16)
            S5B = pool.tile([P, 3 * 16], bf16)
            C5B = pool.tile([P, 3 * 16], bf16)

            repl(wa, NPA, 0, HA)
            k1(wa, NPA, S1A, C1A, CRA, 0, HA)
            repl(wa, NPA, HA, NPA)
            k1(wa, NPA, S1A, C1A, CRA, HA, NPA)
            level(S1A, C1A, NPA, S2A, C2A, CRA)
            repl(wb, NPB, 0, NPB)
            k1(wb, NPB, S1B, C1B, CRB, 0, NPB)
            level(S2A, C2A, 384, S3A, C3A, CRA)
            level(S1B, C1B, NPB, S2B, C2B, CRB)
            level(S3A, C3A, 192, S4A, C4A, CRA)
            level(S2B, C2B, 128, S3B, C3B, CRB)
            level(S4A, C4A, 96, S5A, C5A, CRA, last=True)
            level(S3B, C3B, 64, S4B, C4B, CRB)
            level(S4B, C4B, 32, S5B, C5B, CRB, last=True)

            # ---- Z join: Z = S + (DT/2)*C over all four pieces ----
            # 16-part of Z cols [0,128): [A-ev 48 | B-ev 16 | A-od 48 | B-od 16]
            # 32-part cols [128,192): [A 48 | B 16]
            Z = pool.tile([P, 3 * 192], bf16)
            Z3 = Z.rearrange("p (c n) -> p c n", c=3)

            nc.vector.scalar_tensor_tensor(Z3[:, :, 0:48],
                                           c3v(C4A, 96, 0, 0, 48), DT / 2,
                                           c3v(S4A, 96, 0, 0, 48),
                                           OP.mult, OP.add)
            nc.vector.scalar_tensor_tensor(Z3[:, :, 64:112],
                                           c3v(C4A, 96, 0, 48, 48), DT / 2,
                                           c3v(S4A, 96, 0, 48, 48),
                                           OP.mult, OP.add)
            nc.vector.scalar_tensor_tensor(Z3[:, :, 48:64],
                                           c3v(C4B, 32, 0, 0, 16), DT / 2,
                                           c3v(S4B, 32, 0, 0, 16),
                                           OP.mult, OP.add)
            nc.vector.scalar_tensor_tensor(Z3[:, :, 112:128],
                                           c3v(C4B, 32, 0, 16, 16), DT / 2,
                                           c3v(S4B, 32, 0, 16, 16),
                                           OP.mult, OP.add)
            nc.vector.scalar_tensor_tensor(Z3[:, :, 128:176], C5A[:], DT / 2,
                                           S5A[:], OP.mult, OP.add)
            nc.vector.scalar_tensor_tensor(Z3[:, :, 176:192], C5B[:], DT / 2,
                                           S5B[:], OP.mult, OP.add)

            # ---- tan-poly: gh (5-plane) = tan(|Zd|/2)/|Zd| * Zd, Zd=DT*Z ----
            sqz = pool.tile([P, 3 * 192], bf16)
            nc.vector.tensor_tensor(sqz[:], Z[:], Z[:], OP.mult)
            n2z = pool.tile([P, 192], bf16)
            nc.vector.tensor_tensor(n2z[:], sqz[:, 0:192], sqz[:, 192:384], OP.add)
            nc.vector.tensor_tensor(n2z[:], n2z[:], sqz[:, 384:576], OP.add)
            tp = pool.tile([P, 192], bf16)
            nc.vector.tensor_scalar(tp[:], n2z[:], DT ** 4 / 240, DT ** 2 / 24,
                                    OP.mult, OP.add)
            nc.vector.tensor_tensor(tp[:], tp[:], n2z[:], OP.mult)
            nc.vector.tensor_scalar(tp[:], tp[:], DT, 0.5 * DT, OP.mult, OP.add)
            gh = pool.tile([P, 5 * 192], bf16)
            gh5 = gh.rearrange("p (c n) -> p c n", c=5)
            tpb3 = tp[:].unsqueeze(1).broadcast_to([P, 3, 192])
            tpb2 = tp[:].unsqueeze(1).broadcast_to([P, 2, 192])
            nc.vector.tensor_tensor(gh5[:, 0:3], tpb3, Z3, OP.mult)
            nc.vector.tensor_tensor(gh5[:, 3:5], tpb2, Z3[:, 0:2], OP.mult)

            # ---- d16 exp (unit quats, rsqrt table + Sin table), GpSimd+ACT ----
            # dq: (P, 4*192): [w | x | y | z], each [d16 0:128 | d32 128:192]
            dq = pool.tile([P, 4 * 192], bf16)
            dq4 = dq.rearrange("p (c n) -> p c n", c=4)
            d3 = d.rearrange("p (c j) -> p c j", c=3)
            sqd = pool.tile([P, 3 * NB16], bf16)
            nc.gpsimd.tensor_tensor(sqd[:], d[:], d[:], OP.mult)
            n2d = pool.tile([P, NB16], bf16)
            nc.gpsimd.tensor_tensor(n2d[:], sqd[:, 0:128], sqd[:, 128:256], OP.add)
            nc.gpsimd.tensor_tensor(n2d[:], n2d[:], sqd[:, 256:384], OP.add)
            nc.gpsimd.tensor_scalar(n2d[:], n2d[:], 1e-30, None, OP.max)
            y1 = pool.tile([P, NB16], bf16)
            nc.scalar.activation(y1[:], n2d[:], AF.Abs_reciprocal_sqrt)
            th = pool.tile([P, NB16], bf16)
            nc.gpsimd.tensor_tensor(th[:], n2d[:], y1[:], OP.mult)
            # cos(th/2) = sin(pi/2 - th/2); sin(th/2)
            nc.scalar.activation(dq[:, 0:128], th[:], AF.Sin,
                                 bias=hpi[:], scale=-0.5)
            s0 = pool.tile([P, NB16], bf16)
            nc.scalar.activation(s0[:], th[:], AF.Sin, scale=0.5)
            nc.gpsimd.tensor_tensor(s0[:], s0[:], y1[:], OP.mult)
            s0b = s0[:].unsqueeze(1).broadcast_to([P, 3, NB16])
            nc.gpsimd.tensor_tensor(dq4[:, 1:4, 0:128], s0b, d3, OP.mult)

            # ---- d32 = qmul(d16 evens, d16 odds) on DVE ----
            # evens at 16-cols [0,64), odds [64,128); out 32-cols [128,192)
            q1r = pool.tile([P, 5 * 64], bf16)
            q2r = pool.tile([P, 5 * 64], bf16)
            q1r5 = q1r.rearrange("p (c n) -> p c n", c=5)
            q2r5 = q2r.rearrange("p (c n) -> p c n", c=5)
            nc.vector.tensor_copy(q1r5[:, 0:3], dq4[:, 1:4, 0:64])
            nc.vector.tensor_copy(q1r5[:, 3:5], dq4[:, 1:3, 0:64])
            nc.vector.tensor_copy(q2r5[:, 0:3], dq4[:, 1:4, 64:128])
            nc.vector.tensor_copy(q2r5[:, 3:5], dq4[:, 1:3, 64:128])
            pp = pool.tile([P, 4 * 64], bf16)
            pp4 = pp.rearrange("p (c n) -> p c n", c=4)
            nc.vector.tensor_tensor(pp4, dq4[:, :, 0:64], dq4[:, :, 64:128],
                                    OP.mult)
            w32 = dq[:, 128:192]
            nc.vector.tensor_tensor(w32, pp[:, 0:64], pp[:, 64:128], OP.subtract)
            nc.vector.tensor_tensor(w32, w32, pp[:, 128:192], OP.subtract)
            nc.vector.tensor_tensor(w32, w32, pp[:, 192:256], OP.subtract)
            w1b = dq[:, 0:64].unsqueeze(1).broadcast_to([P, 3, 64])
            w2b = dq[:, 64:128].unsqueeze(1).broadcast_to([P, 3, 64])
            t1 = pool.tile([P, 3 * 64], bf16)
            t13 = t1.rearrange("p (c n) -> p c n", c=3)
            t2 = pool.tile([P, 3 * 64], bf16)
            t23 = t2.rearrange("p (c n) -> p c n", c=3)
            nc.vector.tensor_tensor(t13, w1b, q2r5[:, 0:3], OP.mult)
            nc.vector.tensor_tensor(t23, w2b, q1r5[:, 0:3], OP.mult)
            nc.vector.tensor_tensor(t1[:], t1[:], t2[:], OP.add)
            cr1 = pool.tile([P, 3 * 64], bf16)
            nc.vector.tensor_tensor(cr1.rearrange("p (c n) -> p c n", c=3),
                                    q1r5[:, 1:4], q2r5[:, 2:5], OP.mult)
            nc.vector.tensor_tensor(t23, q1r5[:, 2:5], q2r5[:, 1:4], OP.mult)
            nc.vector.tensor_tensor(cr1[:], cr1[:], t2[:], OP.subtract)
            nc.vector.tensor_tensor(t1[:], t1[:], cr1[:], OP.add)
            nc.vector.tensor_copy(dq4[:, 1:4, 128:192], t13)

            # ---- rel = conj(1, gh) x dq   (width 192), DVE bf16 ----
            dqv = dq4[:, 1:4, :]
            dqr = pool.tile([P, 5 * 192], bf16)
            dqr5 = dqr.rearrange("p (c n) -> p c n", c=5)
            nc.vector.tensor_copy(dqr[:, 0:576], dq[:, 192:768])
            nc.vector.tensor_copy(dqr[:, 576:960], dq[:, 192:576])
            dm = pool.tile([P, 3 * 192], bf16)
            nc.vector.tensor_tensor(dm.rearrange("p (c n) -> p c n", c=3),
                                    gh5[:, 0:3], dqv, OP.mult)
            rw = pool.tile([P, 192], bf16)
            nc.vector.tensor_tensor(rw[:], dm[:, 0:192], dm[:, 192:384], OP.add)
            nc.vector.tensor_tensor(rw[:], rw[:], dm[:, 384:576], OP.add)
            nc.vector.tensor_tensor(rw[:], rw[:], dq[:, 0:192], OP.add)
            cwb = dq[:, 0:192].unsqueeze(1).broadcast_to([P, 3, 192])
            rv = pool.tile([P, 3 * 192], bf16)
            rv3 = rv.rearrange("p (c n) -> p c n", c=3)
            nc.vector.tensor_tensor(rv3, cwb, gh5[:, 0:3], OP.mult)
            nc.vector.tensor_tensor(rv[:], dq[:, 192:], rv[:], OP.subtract)
            crr = pool.tile([P, 3 * 192], bf16)
            nc.vector.tensor_tensor(crr.rearrange("p (c n) -> p c n", c=3),
                                    gh5[:, 1:4], dqr5[:, 2:5], OP.mult)
            nc.vector.tensor_tensor(dm.rearrange("p (c n) -> p c n", c=3),
                                    gh5[:, 2:5], dqr5[:, 1:4], OP.mult)
            nc.vector.tensor_tensor(crr[:], crr[:], dm[:], OP.subtract)
            nc.vector.tensor_tensor(rv[:], rv[:], crr[:], OP.subtract)

            # ---- log (fp32 from squares onward; reference clip semantics) ----
            W2 = 192
            sqv = pool.tile([P, 3 * W2], f32)
            nc.scalar.activation(sqv[:], rv[:], AF.Square)
            n2v = pool.tile([P, W2], f32)
            nc.vector.tensor_tensor(n2v[:], sqv[:, 0:192], sqv[:, 192:384], OP.add)
            nc.vector.tensor_tensor(n2v[:], n2v[:], sqv[:, 384:576], OP.add)
            w2t = pool.tile([P, W2], f32)
            nc.scalar.activation(w2t[:], rw[:], AF.Square)
            q2t = pool.tile([P, W2], f32)
            nc.vector.tensor_tensor(q2t[:], w2t[:], n2v[:], OP.add)
            rq = pool.tile([P, W2], f32)
            nc.vector.reciprocal_approx_fast(rq[:], q2t[:])
            cost = pool.tile([P, W2], f32)
            nc.vector.tensor_tensor(cost[:], w2t[:], n2v[:], OP.subtract)
            nc.vector.tensor_tensor(cost[:], cost[:], rq[:], OP.mult)
            nc.vector.tensor_scalar(cost[:], cost[:], CLP, -CLP, OP.min, OP.max)
            c2t = pool.tile([P, W2], f32)
            nc.scalar.activation(c2t[:], cost[:], AF.Square)
            nc.vector.tensor_scalar(c2t[:], c2t[:], -1.0, 1.0, OP.mult, OP.add)
            rs2 = pool.tile([P, W2], f32)
            nc.scalar.activation(rs2[:], c2t[:], AF.Abs_reciprocal_sqrt)
            # F = 0.5*arccos(c)/sin(arccos(c)): deg-10 poly in t=|c|-1 plus
            # pi/2*rsqrt correction for c<0; coefs folded so e_i = a*k1+k0'.
            KP = [0.5000000010056445, -0.1666664296147386, 0.06667585538901223,
                  -0.028433366986487976, 0.013753622162797092,
                  -0.0011196834360748097, 0.015245614903288171,
                  0.020070084287574758, 0.02282400093211004,
                  0.01299667485963209, 0.0037463467111214254]
            a = pool.tile([P, W2], f32)
            nc.scalar.activation(a[:], cost[:], AF.Abs)
            t2p = pool.tile([P, W2], f32)
            nc.scalar.activation(t2p[:], a[:], AF.Square, bias=m1t[:])
            t4p = pool.tile([P, W2], f32)
            nc.vector.tensor_tensor(t4p[:], t2p[:], t2p[:], OP.mult)
            e0 = pool.tile([P, W2], f32)
            e1 = pool.tile([P, W2], f32)
            e2 = pool.tile([P, W2], f32)
            e3 = pool.tile([P, W2], f32)
            g2 = pool.tile([P, W2], f32)
            nc.vector.tensor_scalar(e0[:], a[:], KP[1], KP[0] - KP[1],
                                    OP.mult, OP.add)
            nc.vector.tensor_scalar(e1[:], a[:], KP[3], KP[2] - KP[3],
                                    OP.mult, OP.add)
            nc.vector.tensor_scalar(e2[:], a[:], KP[5], KP[4] - KP[5],
                                    OP.mult, OP.add)
            nc.vector.tensor_scalar(e3[:], a[:], KP[7], KP[6] - KP[7],
                                    OP.mult, OP.add)
            e4 = pool.tile([P, W2], f32)
            nc.vector.tensor_scalar(e4[:], a[:], KP[9], KP[8] - KP[9],
                                    OP.mult, OP.add)
            nc.vector.scalar_tensor_tensor(g2[:], t2p[:], KP[10], e4[:],
                                           OP.mult, OP.add)
            nc.vector.tensor_tensor(e1[:], e1[:], t2p[:], OP.mult)
            nc.vector.tensor_tensor(e0[:], e0[:], e1[:], OP.add)     # f0
            nc.vector.tensor_tensor(e3[:], e3[:], t2p[:], OP.mult)
            nc.vector.tensor_tensor(e2[:], e2[:], e3[:], OP.add)     # f1
            nc.vector.tensor_tensor(g2[:], g2[:], t4p[:], OP.mult)
            nc.vector.tensor_tensor(g2[:], g2[:], e2[:], OP.add)
            nc.vector.tensor_tensor(g2[:], g2[:], t4p[:], OP.mult)
            nc.vector.tensor_tensor(g2[:], g2[:], e0[:], OP.add)     # p = F(|c|)
            sgn = pool.tile([P, W2], f32)
            nc.scalar.activation(sgn[:], cost[:], AF.Sign)
            u1 = pool.tile([P, W2], f32)
            nc.vector.tensor_scalar(u1[:], sgn[:], float(-np.pi / 4),
                                    float(np.pi / 4), OP.mult, OP.add)
            nc.vector.tensor_tensor(u1[:], u1[:], rs2[:], OP.mult)
            nc.vector.tensor_tensor(g2[:], g2[:], sgn[:], OP.mult)
            cf = pool.tile([P, W2], f32)
            nc.vector.tensor_tensor(cf[:], u1[:], g2[:], OP.add)     # 0.5*ang/sin
            cf2 = pool.tile([P, W2], f32)
            nc.vector.scalar_tensor_tensor(cf2[:], rw[:], 4.0, rq[:],
                                           OP.mult, OP.mult)
            nc.vector.tensor_tensor(cf[:], cf[:], cf2[:], OP.mult)
            rs = pool.tile([P, 3 * W2], f32)
            cfb = cf[:].unsqueeze(1).broadcast_to([P, 3, W2])
            rs3 = rs.rearrange("p (c n) -> p c n", c=3)
            nc.vector.tensor_tensor(rs3, cfb, rv3, OP.mult)

            # ---- skip-block export + huber + partial sums ----
            rs4 = rs.rearrange("p (c g n) -> p c g n", c=3, g=3)  # 64-col groups
            # 16-level skips: logical blocks {0..4} live at cols {0,1,2,64,65}
            nc.sync.dma_start(skip_d[:, 0:9], rs4[0:P:16, :, 0, 0:3])
            nc.sync.dma_start(skip_d[:, 9:18], rs4[0:P:16, :, 1, 0:3])
            nc.sync.dma_start(skip_d[:, 18:33], rs3[0:P:16, :, 128:128 + N0])
            xb = pool.tile([P, 3 * W2], f32)
            nc.scalar.activation(xb[:], rs[:], AF.Abs, scale=1.0 / HUBER)
            mb = pool.tile([P, 3 * W2], f32)
            nc.vector.tensor_scalar(mb[:], xb[:], 1.0, None, OP.min)
            tb = pool.tile([P, 3 * W2], f32)
            nc.vector.scalar_tensor_tensor(tb[:], mb[:], -0.5, xb[:],
                                           OP.mult, OP.add)
            nc.vector.tensor_tensor(tb[:], tb[:], mb[:], OP.mult)
            part = pool.tile([P, 2], f32)
            tb3 = tb.rearrange("p (c n) -> p c n", c=3)
            nc.vector.tensor_reduce(part[:, 0:1], tb3[:, :, 0:128], AX.XY, OP.add)
            nc.vector.tensor_reduce(part[:, 1:2], tb3[:, :, 128:192], AX.XY, OP.add)
            nc.sync.dma_start(o_d[:], part[:])

            if debug:
                for name, t in [("dbg_Z", Z), ("dbg_gh", gh), ("dbg_dq", dq),
                                ("dbg_rw", rw), ("dbg_rv", rv), ("dbg_rs", rs)]:
                    dd = nc.declare_dram_parameter(name, list(t[:].shape),
                                                   t[:].dtype, isOutput=True)
                    nc.sync.dma_start(dd[:], t[:])

    nc.compile()
    return nc


def _get_nc():
    if "nc" not in _CACHE:
        _CACHE["nc"] = _build()
    return _CACHE["nc"]


def _dq16_logical():
    """logical 16-block index for each dq 16-part column j in [0,128)."""
    j = np.arange(NB16)
    lb = np.empty(NB16, dtype=np.int64)
    aev = j < 48
    bev = (j >= 48) & (j < 64)
    aod = (j >= 64) & (j < 112)
    bod = j >= 112
    lb[aev] = 2 * j[aev]
    lb[bev] = NBA + 2 * (j[bev] - 48)
    lb[aod] = 2 * (j[aod] - 64) + 1
    lb[bod] = NBA + 2 * (j[bod] - 112) + 1
    return lb


def shard_inputs(w_hat, dw_16):
    """full inputs -> list of per-core {'w','d'} maps (permuted bf16 planes)."""
    import ml_dtypes
    bf = ml_dtypes.bfloat16
    posA = _pair_pos(NBA)
    posB = _pair_pos(NBB)
    invA = np.empty_like(posA); invA[posA] = np.arange(NPA)
    invB = np.empty_like(posB); invB[posB] = np.arange(NPB)
    dperm = _dq16_logical()
    maps = []
    for c in range(NCORES):
        wc = w_hat[c * SPC:(c + 1) * SPC].reshape(P, STEPS // 2, 2, 3)
        ev, od = wc[:, :, 0], wc[:, :, 1]      # (P, 1024, 3)
        evA, odA = ev[:, 0:NPA][:, invA], od[:, 0:NPA][:, invA]
        evB, odB = ev[:, NPA:][:, invB], od[:, NPA:][:, invB]
        # planes: [ev x y z | od x y z] per region, digit-rev pair order
        wa = np.concatenate([evA.transpose(0, 2, 1), odA.transpose(0, 2, 1)], 1)
        wbp = np.concatenate([evB.transpose(0, 2, 1), odB.transpose(0, 2, 1)], 1)
        w = np.concatenate([wa.reshape(P, 6 * NPA), wbp.reshape(P, 6 * NPB)], 1)
        dc = dw_16[c * SPC:(c + 1) * SPC, ::16].reshape(P, NB16, 3)
        dc = np.ascontiguousarray(dc[:, dperm]).transpose(0, 2, 1)  # planar
        maps.append({"w": np.ascontiguousarray(w).astype(bf),
                     "d": np.ascontiguousarray(dc).reshape(P, 3 * NB16).astype(bf)})
    return maps


def _huber_sum_f32(rs_flat):
    """Same f32 ops as the device huber."""
    x = (np.abs(rs_flat) * np.float32(1.0 / HUBER)).astype(np.float32)
    m = np.minimum(x, np.float32(1.0))
    t = (m * np.float32(-0.5) + x).astype(np.float32)
    return (m * t).astype(np.float32).sum(dtype=np.float64)


def combine_outputs(outs):
    """list of per-core {'out', 'skip'} -> scalar loss (np.float32)."""
    s16 = 0.0
    s32 = 0.0
    for om in outs:
        o = np.asarray(om["out"], dtype=np.float64)
        s16 += o[:, 0].sum()
        s32 += o[:, 1].sum()
        sk = np.asarray(om["skip"], dtype=np.float32)
        g1 = sk[:, 9:18].reshape(SPC, 3, 3)
        # cols {0,1,2} = logical {0,2,4}; cols {64,65} = logical {1,3}
        sel = np.concatenate([sk[:, 0:9], g1[:, :, 0:2].reshape(SPC, -1)], 1)
        s16 -= _huber_sum_f32(sel)
        s32 -= _huber_sum_f32(sk[:, 18:33])
    c16 = NSEQ * (T // 16 - N0) * 3
    c32 = NSEQ * (T // 32 - N0) * 3
    loss = WLOSS * HUBER ** 2 * (s16 / c16) + WLOSS * HUBER ** 2 * (s32 / c32) / 4.0
    return np.float32(loss)


def kernel(w_hat, dw_16):
    from concourse.bass_utils import run_bass_kernel_spmd

    w_hat = np.asarray(w_hat, dtype=np.float32)
    dw_16 = np.asarray(dw_16, dtype=np.float32)
    nc = _get_nc()
    in_maps = shard_inputs(w_hat, dw_16)
    res = run_bass_kernel_spmd(nc, in_maps, list(range(NCORES)))
    return combine_outputs(res.results)


# revision 7
# speedup vs baseline: 1.7977x; 1.2725x over previous
"""Trainium2 Bass kernel for nn_DGLossVer1 (SO(3) gyro loss), bf16 edition.

Math identical to the fp32 baseline: product of 16 (or 32) small-rotation
exponentials via 2nd-order BCH (Z = dt*S + dt^2/2 * C, pairwise tree with
C_AB = C_A + C_B + S_A x S_B), block rotation as unnormalized quaternion
(1, tan(|Z|/2)/|Z| * Z); GT side as true unit quats via rsqrt + Sin table;
rs = log(conj(pred) x gt) with the reference clip semantics.

Perf design:
- Single-region digit-reversed tree, entirely on DVE in bf16 (2x TT mode);
  x,y replica planes built on-chip with TensorCopy. The ~400K-term mean
  absorbs bf16's 0.4% per-value noise (measured end error ~4e-4).
- 3 input DMAs + all output DMAs issued from GpSimd (25ns issue vs 565ns
  on Sync), inputs in 2 chunks so k1 starts during the DMA.
- qmul32 + rel on DVE; GT-exp side on GpSimd+ACT with the
  Abs_reciprocal_sqrt table (4e-5 max rel err; eps guard folded into the
  activation bias); 1/q^2 via reciprocal_approx_fast.
- log phase fp32 from the squares onward (clip + 1-cos^2 need it); the
  0.5*acos(c)/sin coefficient uses a deg-5 Horner fit in |c| (1.7e-5 rel).

Sharding: pure data parallel, 8 sequences per core; each core returns two
partial Huber sums per partition plus the skipped-block rs values; the
host does the tiny weighted reduction (and subtracts the N0 skips).
"""
import numpy as np

P = 128
DT = 0.005
WLOSS = 1.0e6
HUBER = 0.005
N0 = 5
NSEQ, T = 64, 32768
NCORES = 8
SPC = NSEQ // NCORES          # sequences per core
STEPS = SPC * T // P          # 2048 steps per partition
NB16 = STEPS // 16            # 128 16-blocks per partition
NP = STEPS // 2               # 1024 step-pairs per partition
WCOLS = 6 * NP                # host planes: ev x,y,z | od x,y,z (6144)
SKW = 6 * 3 + 5 * 3           # skip outputs per sequence (33)

_CACHE = {}


def _pair_pos(nb):
    """digit-reversed position of logical pair i (n = nb*8)."""
    i = np.arange(nb * 8)
    t = i % 8
    B = i // 8
    t1, t2, t3 = t & 1, (t >> 1) & 1, (t >> 2) & 1
    return (t1 * 4 + t2 * 2 + t3) * nb + (B % 2) * (nb // 2) + B // 2


def _sigma16():
    """logical 16-block index held by dq 16-part column j in [0,128)."""
    pos = _pair_pos(NB16)
    state = np.empty(NP, dtype=np.int64)
    state[pos] = np.arange(NP)
    for _ in range(3):
        state = state[:len(state) // 2] // 2
    return state  # cols of logical {0..4}: 0, 64, 1, 65, 2


def _build(debug=False):
    import concourse.bass as bass
    import concourse.tile as tile
    import concourse.mybir as mybir
    from concourse import bacc

    f32 = mybir.dt.float32
    bf16 = mybir.dt.bfloat16
    AF = mybir.ActivationFunctionType
    OP = mybir.AluOpType
    AX = mybir.AxisListType

    nc = bacc.Bacc(None)
    w_d = nc.declare_dram_parameter("w", [P, WCOLS], bf16, isOutput=False)
    d_d = nc.declare_dram_parameter("d", [P, 3 * NB16], bf16, isOutput=False)
    o_d = nc.declare_dram_parameter("out", [P, 2], f32, isOutput=True)
    skip_d = nc.declare_dram_parameter("skip", [SPC, SKW], f32, isOutput=True)

    CLP = 1.0 - 1e-7

    with tile.TileContext(nc) as tc:
        with tc.tile_pool(name="main", bufs=1) as pool:
            # ---- input DMA: d, then w in 2 half-chunks (GpSimd issue) ----
            wa = pool.tile([P, 10 * NP], bf16)
            d = pool.tile([P, 3 * NB16], bf16)
            wa10 = wa.rearrange("p (k n) -> p k n", k=10)
            wd6 = w_d.rearrange("p (k n) -> p k n", k=6)
            H = NP // 2
            nc.gpsimd.dma_start(d[:], d_d[:])
            nc.gpsimd.dma_start(wa10[:, 0:3, 0:H], wd6[:, 0:3, 0:H])
            nc.gpsimd.dma_start(wa10[:, 5:8, 0:H], wd6[:, 3:6, 0:H])
            nc.gpsimd.dma_start(wa10[:, 0:3, H:NP], wd6[:, 0:3, H:NP])
            nc.gpsimd.dma_start(wa10[:, 5:8, H:NP], wd6[:, 3:6, H:NP])

            hpi = pool.tile([P, 1], f32)
            nc.gpsimd.memset(hpi[:], float(np.pi / 2))
            epsb = pool.tile([P, 1], f32)
            nc.gpsimd.memset(epsb[:], 1e-30)

            def c3v(t, n, block, off, cnt, nb=3):
                nblocks = t[:].shape[1] // n
                return t.rearrange("p (k n) -> p k n", k=nblocks)[
                    :, block:block + nb, off:off + cnt]

            def repl(wt, lo, hi):
                w10 = wt.rearrange("p (k n) -> p k n", k=10)
                nc.vector.tensor_copy(w10[:, 3:5, lo:hi], w10[:, 0:2, lo:hi])
                nc.vector.tensor_copy(w10[:, 8:10, lo:hi], w10[:, 5:7, lo:hi])

            def k1(wt, npr, S1, C1, CR, lo, hi):
                cnt = hi - lo
                m1 = c3v(C1, npr, 0, lo, cnt)
                nc.vector.tensor_tensor(m1, c3v(wt, npr, 1, lo, cnt),
                                        c3v(wt, npr, 7, lo, cnt), OP.mult)
                m2 = c3v(CR, npr, 0, lo, cnt)
                nc.vector.tensor_tensor(m2, c3v(wt, npr, 2, lo, cnt),
                                        c3v(wt, npr, 6, lo, cnt), OP.mult)
                nc.vector.tensor_tensor(m1, m1, m2, OP.subtract)
                nc.vector.tensor_tensor(c3v(S1, npr, 0, lo, cnt),
                                        c3v(wt, npr, 0, lo, cnt),
                                        c3v(wt, npr, 5, lo, cnt), OP.add)
                nc.vector.tensor_copy(c3v(S1, npr, 3, lo, cnt, 2),
                                      c3v(S1, npr, 0, lo, cnt, 2))

            def level(Sp, Cp, n_in, Sn, Cn, CR, last=False):
                n = n_in // 2
                m1 = c3v(Cn, n, 0, 0, n)
                nc.vector.tensor_tensor(m1, c3v(Sp, n_in, 1, 0, n),
                                        c3v(Sp, n_in, 2, n, n), OP.mult)
                m2 = c3v(CR, n, 0, 0, n)
                nc.vector.tensor_tensor(m2, c3v(Sp, n_in, 2, 0, n),
                                        c3v(Sp, n_in, 1, n, n), OP.mult)
                nc.vector.tensor_tensor(m1, m1, m2, OP.subtract)
                nc.vector.tensor_tensor(m2, c3v(Cp, n_in, 0, 0, n),
                                        c3v(Cp, n_in, 0, n, n), OP.add)
                nc.vector.tensor_tensor(m1, m1, m2, OP.add)
                nc.vector.tensor_tensor(c3v(Sn, n, 0, 0, n),
                                        c3v(Sp, n_in, 0, 0, n),
                                        c3v(Sp, n_in, 0, n, n), OP.add)
                if not last:
                    nc.vector.tensor_copy(c3v(Sn, n, 3, 0, n, 2),
                                          c3v(Sn, n, 0, 0, n, 2))

            S1 = pool.tile([P, 5 * NP], bf16)
            C1 = pool.tile([P, 3 * NP], bf16)
            CR = pool.tile([P, 3 * NP], bf16)
            S2 = pool.tile([P, 5 * 512], bf16)
            C2 = pool.tile([P, 3 * 512], bf16)
            S3 = pool.tile([P, 5 * 256], bf16)
            C3 = pool.tile([P, 3 * 256], bf16)
            S4 = pool.tile([P, 5 * 128], bf16)
            C4 = pool.tile([P, 3 * 128], bf16)
            S5 = pool.tile([P, 3 * 64], bf16)
            C5 = pool.tile([P, 3 * 64], bf16)

            repl(wa, 0, H)
            k1(wa, NP, S1, C1, CR, 0, H)
            repl(wa, H, NP)
            k1(wa, NP, S1, C1, CR, H, NP)
            level(S1, C1, NP, S2, C2, CR)
            level(S2, C2, 512, S3, C3, CR)
            level(S3, C3, 256, S4, C4, CR)
            level(S4, C4, 128, S5, C5, CR, last=True)

            # ---- Z join: 16-part cols [0,128), 32-part cols [128,192) ----
            Z = pool.tile([P, 3 * 192], bf16)
            Z3 = Z.rearrange("p (c n) -> p c n", c=3)
            nc.vector.scalar_tensor_tensor(Z3[:, :, 0:128],
                                           c3v(C4, 128, 0, 0, 128), DT / 2,
                                           c3v(S4, 128, 0, 0, 128),
                                           OP.mult, OP.add)
            nc.vector.scalar_tensor_tensor(Z3[:, :, 128:192], C5[:], DT / 2,
                                           S5[:], OP.mult, OP.add)

            # ---- tan-poly: gh (5-plane) ----
            sqz = pool.tile([P, 3 * 192], bf16)
            nc.vector.tensor_tensor(sqz[:], Z[:], Z[:], OP.mult)
            n2z = pool.tile([P, 192], bf16)
            nc.vector.tensor_tensor(n2z[:], sqz[:, 0:192], sqz[:, 192:384], OP.add)
            nc.vector.tensor_tensor(n2z[:], n2z[:], sqz[:, 384:576], OP.add)
            tp = pool.tile([P, 192], bf16)
            nc.vector.tensor_scalar(tp[:], n2z[:], DT ** 4 / 240, DT ** 2 / 24,
                                    OP.mult, OP.add)
            nc.vector.tensor_tensor(tp[:], tp[:], n2z[:], OP.mult)
            nc.vector.tensor_scalar(tp[:], tp[:], DT, 0.5 * DT, OP.mult, OP.add)
            gh = pool.tile([P, 5 * 192], bf16)
            gh5 = gh.rearrange("p (c n) -> p c n", c=5)
            tpb3 = tp[:].unsqueeze(1).broadcast_to([P, 3, 192])
            tpb2 = tp[:].unsqueeze(1).broadcast_to([P, 2, 192])
            nc.vector.tensor_tensor(gh5[:, 0:3], tpb3, Z3, OP.mult)
            nc.vector.tensor_tensor(gh5[:, 3:5], tpb2, Z3[:, 0:2], OP.mult)

            # ---- d16 exp (rsqrt table + Sin table), GpSimd+ACT ----
            dq = pool.tile([P, 4 * 192], bf16)
            dq4 = dq.rearrange("p (c n) -> p c n", c=4)
            d3 = d.rearrange("p (c j) -> p c j", c=3)
            sqd = pool.tile([P, 3 * NB16], bf16)
            nc.gpsimd.tensor_tensor(sqd[:], d[:], d[:], OP.mult)
            n2d = pool.tile([P, NB16], bf16)
            nc.gpsimd.tensor_tensor(n2d[:], sqd[:, 0:128], sqd[:, 128:256], OP.add)
            nc.gpsimd.tensor_tensor(n2d[:], n2d[:], sqd[:, 256:384], OP.add)
            y1 = pool.tile([P, NB16], bf16)
            nc.scalar.activation(y1[:], n2d[:], AF.Abs_reciprocal_sqrt, bias=epsb[:])
            th = pool.tile([P, NB16], bf16)
            nc.gpsimd.tensor_tensor(th[:], n2d[:], y1[:], OP.mult)
            nc.scalar.activation(dq[:, 0:128], th[:], AF.Sin, bias=hpi[:], scale=-0.5)
            s0 = pool.tile([P, NB16], bf16)
            nc.scalar.activation(s0[:], th[:], AF.Sin, scale=0.5)
            nc.gpsimd.tensor_tensor(s0[:], s0[:], y1[:], OP.mult)
            s0b = s0[:].unsqueeze(1).broadcast_to([P, 3, NB16])
            nc.gpsimd.tensor_tensor(dq4[:, 1:4, 0:128], s0b, d3, OP.mult)

            # ---- d32 = qmul(d16 evens, d16 odds) on DVE ----
            q1r = pool.tile([P, 5 * 64], bf16)
            q2r = pool.tile([P, 5 * 64], bf16)
            q1r5 = q1r.rearrange("p (c n) -> p c n", c=5)
            q2r5 = q2r.rearrange("p (c n) -> p c n", c=5)
            nc.vector.tensor_copy(q1r5[:, 0:3], dq4[:, 1:4, 0:64])
            nc.vector.tensor_copy(q1r5[:, 3:5], dq4[:, 1:3, 0:64])
            nc.vector.tensor_copy(q2r5[:, 0:3], dq4[:, 1:4, 64:128])
            nc.vector.tensor_copy(q2r5[:, 3:5], dq4[:, 1:3, 64:128])
            pp = pool.tile([P, 4 * 64], bf16)
            pp4 = pp.rearrange("p (c n) -> p c n", c=4)
            nc.vector.tensor_tensor(pp4, dq4[:, :, 0:64], dq4[:, :, 64:128],
                                    OP.mult)
            w32 = dq[:, 128:192]
            nc.vector.tensor_tensor(w32, pp[:, 0:64], pp[:, 64:128], OP.subtract)
            nc.vector.tensor_tensor(w32, w32, pp[:, 128:192], OP.subtract)
            nc.vector.tensor_tensor(w32, w32, pp[:, 192:256], OP.subtract)
            w1b = dq[:, 0:64].unsqueeze(1).broadcast_to([P, 3, 64])
            w2b = dq[:, 64:128].unsqueeze(1).broadcast_to([P, 3, 64])
            t1 = pool.tile([P, 3 * 64], bf16)
            t13 = t1.rearrange("p (c n) -> p c n", c=3)
            t2 = pool.tile([P, 3 * 64], bf16)
            t23 = t2.rearrange("p (c n) -> p c n", c=3)
            nc.vector.tensor_tensor(t13, w1b, q2r5[:, 0:3], OP.mult)
            nc.vector.tensor_tensor(t23, w2b, q1r5[:, 0:3], OP.mult)
            nc.vector.tensor_tensor(t1[:], t1[:], t2[:], OP.add)
            cr1 = pool.tile([P, 3 * 64], bf16)
            nc.vector.tensor_tensor(cr1.rearrange("p (c n) -> p c n", c=3),
                                    q1r5[:, 1:4], q2r5[:, 2:5], OP.mult)
            nc.vector.tensor_tensor(t23, q1r5[:, 2:5], q2r5[:, 1:4], OP.mult)
            nc.vector.tensor_tensor(cr1[:], cr1[:], t2[:], OP.subtract)
            nc.vector.tensor_tensor(t1[:], t1[:], cr1[:], OP.add)
            nc.vector.tensor_copy(dq4[:, 1:4, 128:192], t13)

            # ---- rel = conj(1, gh) x dq   (width 192), DVE bf16 ----
            dqv = dq4[:, 1:4, :]
            dqr = pool.tile([P, 5 * 192], bf16)
            dqr5 = dqr.rearrange("p (c n) -> p c n", c=5)
            nc.vector.tensor_copy(dqr[:, 0:576], dq[:, 192:768])
            nc.vector.tensor_copy(dqr[:, 576:960], dq[:, 192:576])
            dm = pool.tile([P, 3 * 192], bf16)
            nc.vector.tensor_tensor(dm.rearrange("p (c n) -> p c n", c=3),
                                    gh5[:, 0:3], dqv, OP.mult)
            rw = pool.tile([P, 192], bf16)
            nc.vector.tensor_tensor(rw[:], dm[:, 0:192], dm[:, 192:384], OP.add)
            nc.vector.tensor_tensor(rw[:], rw[:], dm[:, 384:576], OP.add)
            nc.vector.tensor_tensor(rw[:], rw[:], dq[:, 0:192], OP.add)
            cwb = dq[:, 0:192].unsqueeze(1).broadcast_to([P, 3, 192])
            rv = pool.tile([P, 3 * 192], bf16)
            rv3 = rv.rearrange("p (c n) -> p c n", c=3)
            nc.vector.tensor_tensor(rv3, cwb, gh5[:, 0:3], OP.mult)
            nc.vector.tensor_tensor(rv[:], dq[:, 192:], rv[:], OP.subtract)
            crr = pool.tile([P, 3 * 192], bf16)
            nc.vector.tensor_tensor(crr.rearrange("p (c n) -> p c n", c=3),
                                    gh5[:, 1:4], dqr5[:, 2:5], OP.mult)
            nc.vector.tensor_tensor(dm.rearrange("p (c n) -> p c n", c=3),
                                    gh5[:, 2:5], dqr5[:, 1:4], OP.mult)
            nc.vector.tensor_tensor(crr[:], crr[:], dm[:], OP.subtract)
            nc.vector.tensor_tensor(rv[:], rv[:], crr[:], OP.subtract)

            # ---- log (fp32 from squares onward) ----
            W2 = 192
            sqv = pool.tile([P, 3 * W2], f32)
            nc.scalar.activation(sqv[:], rv[:], AF.Square)
            n2v = pool.tile([P, W2], f32)
            nc.vector.tensor_tensor(n2v[:], sqv[:, 0:192], sqv[:, 192:384], OP.add)
            nc.vector.tensor_tensor(n2v[:], n2v[:], sqv[:, 384:576], OP.add)
            w2t = pool.tile([P, W2], f32)
            nc.scalar.activation(w2t[:], rw[:], AF.Square)
            q2t = pool.tile([P, W2], f32)
            nc.vector.tensor_tensor(q2t[:], w2t[:], n2v[:], OP.add)
            rq = pool.tile([P, W2], f32)
            nc.vector.reciprocal_approx_fast(rq[:], q2t[:])
            cost = pool.tile([P, W2], f32)
            nc.vector.tensor_tensor(cost[:], w2t[:], n2v[:], OP.subtract)
            nc.vector.tensor_tensor(cost[:], cost[:], rq[:], OP.mult)
            nc.vector.tensor_scalar(cost[:], cost[:], CLP, -CLP, OP.min, OP.max)
            c2t = pool.tile([P, W2], f32)
            nc.scalar.activation(c2t[:], cost[:], AF.Square)
            nc.vector.tensor_scalar(c2t[:], c2t[:], -1.0, 1.0, OP.mult, OP.add)
            rs2 = pool.tile([P, W2], f32)
            nc.scalar.activation(rs2[:], c2t[:], AF.Abs_reciprocal_sqrt)
            # F(|c|) = 0.5*acos(|c|)/sin(acos(|c|)): deg-5 Horner in a=|c|
            K5 = [0.7853849420235615, -0.49900465988902176, 0.3800535808218428,
                  -0.2708563016962799, 0.1378553128516594, -0.033441262473293]
            a = pool.tile([P, W2], f32)
            nc.scalar.activation(a[:], cost[:], AF.Abs)
            g2 = pool.tile([P, W2], f32)
            nc.vector.tensor_scalar(g2[:], a[:], K5[5], None, OP.mult)
            for j in (4, 3, 2, 1):
                nc.vector.scalar_tensor_tensor(g2[:], g2[:], K5[j], a[:],
                                               OP.add, OP.mult)
            nc.vector.tensor_scalar(g2[:], g2[:], K5[0], None, OP.add)
            sgn = pool.tile([P, W2], f32)
            nc.scalar.activation(sgn[:], cost[:], AF.Sign)
            u1 = pool.tile([P, W2], f32)
            nc.vector.tensor_scalar(u1[:], sgn[:], float(-np.pi / 4),
                                    float(np.pi / 4), OP.mult, OP.add)
            nc.vector.tensor_tensor(u1[:], u1[:], rs2[:], OP.mult)
            nc.vector.tensor_tensor(g2[:], g2[:], sgn[:], OP.mult)
            cf = pool.tile([P, W2], f32)
            nc.vector.tensor_tensor(cf[:], u1[:], g2[:], OP.add)
            cf2 = pool.tile([P, W2], f32)
            nc.vector.scalar_tensor_tensor(cf2[:], rw[:], 4.0, rq[:],
                                           OP.mult, OP.mult)
            nc.vector.tensor_tensor(cf[:], cf[:], cf2[:], OP.mult)
            rs = pool.tile([P, 3 * W2], f32)
            cfb = cf[:].unsqueeze(1).broadcast_to([P, 3, W2])
            rs3 = rs.rearrange("p (c n) -> p c n", c=3)
            nc.vector.tensor_tensor(rs3, cfb, rv3, OP.mult)

            # ---- skip export + huber + partial sums ----
            rs4 = rs.rearrange("p (c g n) -> p c g n", c=3, g=3)
            nc.gpsimd.dma_start(skip_d[:, 0:9], rs4[0:P:16, :, 0, 0:3])
            nc.gpsimd.dma_start(skip_d[:, 9:18], rs4[0:P:16, :, 1, 0:3])
            nc.gpsimd.dma_start(skip_d[:, 18:33], rs3[0:P:16, :, 128:128 + N0])
            xb = pool.tile([P, 3 * W2], f32)
            nc.scalar.activation(xb[:], rs[:], AF.Abs, scale=1.0 / HUBER)
            mb = pool.tile([P, 3 * W2], f32)
            nc.vector.tensor_scalar(mb[:], xb[:], 1.0, None, OP.min)
            tb = pool.tile([P, 3 * W2], f32)
            nc.vector.scalar_tensor_tensor(tb[:], mb[:], -0.5, xb[:],
                                           OP.mult, OP.add)
            nc.vector.tensor_tensor(tb[:], tb[:], mb[:], OP.mult)
            part = pool.tile([P, 2], f32)
            tb3 = tb.rearrange("p (c n) -> p c n", c=3)
            nc.vector.tensor_reduce(part[:, 0:1], tb3[:, :, 0:128], AX.XY, OP.add)
            nc.vector.tensor_reduce(part[:, 1:2], tb3[:, :, 128:192], AX.XY, OP.add)
            nc.gpsimd.dma_start(o_d[:], part[:])

            if debug:
                for name, t in [("dbg_Z", Z), ("dbg_gh", gh), ("dbg_dq", dq),
                                ("dbg_rw", rw), ("dbg_rv", rv), ("dbg_rs", rs)]:
                    dd = nc.declare_dram_parameter(name, list(t[:].shape),
                                                   t[:].dtype, isOutput=True)
                    nc.sync.dma_start(dd[:], t[:])

    nc.compile()
    return nc


def _get_nc():
    if "nc" not in _CACHE:
        _CACHE["nc"] = _build()
    return _CACHE["nc"]


def shard_inputs(w_hat, dw_16):
    """full inputs -> list of per-core {'w','d'} maps (permuted bf16 planes)."""
    import ml_dtypes
    bf = ml_dtypes.bfloat16
    pos = _pair_pos(NB16)
    inv = np.empty_like(pos); inv[pos] = np.arange(NP)
    dperm = _sigma16()
    maps = []
    for c in range(NCORES):
        wc = w_hat[c * SPC:(c + 1) * SPC].reshape(P, NP, 2, 3)
        ev, od = wc[:, :, 0][:, inv], wc[:, :, 1][:, inv]   # (P, 1024, 3)
        w = np.concatenate([ev.transpose(0, 2, 1).reshape(P, 3 * NP),
                            od.transpose(0, 2, 1).reshape(P, 3 * NP)], 1)
        dc = dw_16[c * SPC:(c + 1) * SPC, ::16].reshape(P, NB16, 3)
        dc = np.ascontiguousarray(dc[:, dperm]).transpose(0, 2, 1)
        maps.append({"w": np.ascontiguousarray(w).astype(bf),
                     "d": np.ascontiguousarray(dc).reshape(P, 3 * NB16).astype(bf)})
    return maps


def _huber_sum_f32(rs_flat):
    x = (np.abs(rs_flat) * np.float32(1.0 / HUBER)).astype(np.float32)
    m = np.minimum(x, np.float32(1.0))
    t = (m * np.float32(-0.5) + x).astype(np.float32)
    return (m * t).astype(np.float32).sum(dtype=np.float64)


def combine_outputs(outs):
    s16 = 0.0
    s32 = 0.0
    for om in outs:
        o = np.asarray(om["out"], dtype=np.float64)
        s16 += o[:, 0].sum()
        s32 += o[:, 1].sum()
        sk = np.asarray(om["skip"], dtype=np.float32)
        g1 = sk[:, 9:18].reshape(SPC, 3, 3)
        # cols {0,1,2} = logical {0,2,4}; cols {64,65} = logical {1,3}
        sel = np.concatenate([sk[:, 0:9], g1[:, :, 0:2].reshape(SPC, -1)], 1)
        s16 -= _huber_sum_f32(sel)
        s32 -= _huber_sum_f32(sk[:, 18:33])
    c16 = NSEQ * (T // 16 - N0) * 3
    c32 = NSEQ * (T // 32 - N0) * 3
    loss = WLOSS * HUBER ** 2 * (s16 / c16) + WLOSS * HUBER ** 2 * (s32 / c32) / 4.0
    return np.float32(loss)


def kernel(w_hat, dw_16):
    from concourse.bass_utils import run_bass_kernel_spmd

    w_hat = np.asarray(w_hat, dtype=np.float32)
    dw_16 = np.asarray(dw_16, dtype=np.float32)
    nc = _get_nc()
    in_maps = shard_inputs(w_hat, dw_16)
    res = run_bass_kernel_spmd(nc, in_maps, list(range(NCORES)))
    return combine_outputs(res.results)
